# revision 3
# baseline (speedup 1.0000x reference)
"""LSTM (BaseRNN) Trainium2 kernel, v4.

Problem: B=128, T=512, I=256, H=768 LSTM; returns (hiddenStates, cellStates)
each [B, T, H] fp32.

Data-parallel over batch (8 cores x 16 rows). v4 structure:

  - xW is precomputed in 8-step blocks (stationary = 8 steps x 16 batch =
    128 wide, streaming W once per 8 steps) but the blocks are emitted
    INLINE in the recurrence loop, 32 steps ahead: their h-independent
    matmuls fill the PE's chain-wait gaps instead of forming a serial
    prologue. Block results round-trip through an Internal DRAM scratch
    (per-step records, DMA does the partition remap) and are prefetched
    into SBUF 8 steps ahead.
  - Per step, xw_t is injected into each half's psum with one
    scatter-matmul (0/1 routing stationary), then 24 U quad-matmuls
    accumulate h U on top.
  - SEPARATE psum tiles per half so each half's sigmoid fires at its own
    accumulation stop.
  - v1-style transposed tail: o' is PE-transposed right after the sigmoid
    (off the critical chain); after tanh, tanh(c)^T is PE-transposed and
    h^T = o'^T * tanh(c)^T is written by one DVE mul straight into the
    next step's stationary tile. Half1's tail is deferred into the next
    step's PE stream.
  - All chain tensor ops on DVE (GPSIMD tensor ops measured ~3x slower);
    ACT does sigmoid/tanh only.
"""

import numpy as np

import concourse.bass as bass
import concourse.bacc as bacc
import concourse.tile as tile
from concourse import mybir
from concourse.bass_utils import run_bass_kernel_spmd
from concourse.masks import make_identity

B, T, I, H = 128, 512, 256, 768
NCORES = 8
NB = B // NCORES  # 16
KX = I // 128  # 2 x chunks
KH = H // 128  # 6 h chunks
NHALF = H // 2  # 384
F32 = mybir.dt.float32
F16 = mybir.dt.float16
XBLK = 32  # x-stream block (steps per DMA)
ABLK = 8  # steps per xW GEMM block
LOOK = 32  # xW block lookahead (steps)
PF = 8  # xw prefetch depth (steps)


def build_lstm(nb=NB, t_steps=T, has_b=False):
    nc = bacc.Bacc(None, target_bir_lowering=False)

    xT_d = nc.dram_tensor("xT", [t_steps, KX, 128, nb], F32, kind="ExternalInput")
    h0_d = nc.dram_tensor("h0", [nb, H], F32, kind="ExternalInput")
    c0_d = nc.dram_tensor("c0", [nb, 2, NHALF], F16, kind="ExternalInput")
    w_d = nc.dram_tensor("w", [KX, 128, 4 * H], F32, kind="ExternalInput")
    u_d = nc.dram_tensor("u", [KH, 128, 4 * H], F32, kind="ExternalInput")
    b_d = nc.dram_tensor("b", [1, 4 * H], F32, kind="ExternalInput")
    hs_d = nc.dram_tensor("hs", [t_steps, 128, KH * NB], F16, kind="ExternalOutput")
    cs_d = nc.dram_tensor("cs", [nb, t_steps, 2, NHALF], F16, kind="ExternalOutput")
    xw_d = nc.dram_tensor("xw", [t_steps, 4, nb, 2, NHALF], F16, kind="Internal")

    SIG = mybir.ActivationFunctionType.Sigmoid
    TANH = mybir.ActivationFunctionType.Tanh
    MULT = mybir.AluOpType.mult
    ADD = mybir.AluOpType.add

    with tile.TileContext(nc) as tc:
        with (
            tc.tile_pool(name="consts", bufs=1) as consts,
            tc.tile_pool(name="xs", bufs=2) as xs_pool,
            tc.tile_pool(name="gsb", bufs=3) as gsb,
            tc.tile_pool(name="ew", bufs=3) as ew,
            tc.tile_pool(name="state", bufs=3) as state,
            tc.tile_pool(name="pg", bufs=2, space="PSUM") as pg,
            tc.tile_pool(name="pa", bufs=1, space="PSUM") as pa,
            tc.tile_pool(name="pt", bufs=2, space="PSUM") as pt,
        ):
            ident16 = consts.tile([nb, nb], F16)
            make_identity(nc, ident16)
            idento = consts.tile([64 + nb, nb], F16)
            make_identity(nc, idento[64 : 64 + nb])
            # injection router: dense xw partitions 16q+b -> psum 32q+b
            sinj_np = np.zeros((4 * nb, 112), np.float16)
            for q in range(4):
                for bb in range(nb):
                    sinj_np[16 * q + bb, 32 * q + bb] = 1.0
            sinj_d = nc.inline_tensor(sinj_np, name="sinj_const")
            sinj = consts.tile([4 * nb, 112], F16)
            nc.sync.dma_start(out=sinj, in_=sinj_d[:, :])
            # per-partition activation input scale: 1 for i/f/o, 2 for g~
            sc = consts.tile([112, 1], F32)
            nc.vector.memset(sc[0:96], 1.0)
            nc.vector.memset(sc[96:112], 2.0)

            w_sb = consts.tile([128, KX, 4 * H], F16)
            u_sb = consts.tile([128, KH, 4 * H], F16)
            for k in range(KX):
                stg = xs_pool.tile([128, 4 * H], F32, tag="WSTG")
                nc.sync.dma_start(out=stg, in_=w_d[k, :, :])
                nc.vector.tensor_copy(w_sb[:, k, :], stg)
            for k in range(KH):
                stg = xs_pool.tile([128, 4 * H], F32, tag="WSTG")
                nc.sync.dma_start(out=stg, in_=u_d[k, :, :])
                nc.vector.tensor_copy(u_sb[:, k, :], stg)
            if has_b:
                b_sb = consts.tile([1, 4 * H], F32)
                nc.sync.dma_start(out=b_sb, in_=b_d[:, :])

            # ---- x staging + inline xW blocks ----
            x_holder = {}

            def x_stage(t0):
                # DMA + f16-convert x for steps [t0, t0+XBLK)
                nblk = min(XBLK, t_steps - t0)
                xstg = xs_pool.tile([128, KX, XBLK, nb], F32, tag="XSTG")
                for k in range(KX):
                    nc.sync.dma_start(
                        out=xstg[:, k, 0:nblk],
                        in_=xT_d[t0 : t0 + nblk, k].rearrange("t p b -> p t b"),
                    )
                xt = xs_pool.tile([128, KX, XBLK, nb], F16, tag="X")
                nc.vector.tensor_copy(xt[:, :, 0:nblk], xstg[:, :, 0:nblk])
                x_holder[t0 // XBLK] = xt

            cp_i = [0]

            def xw_block(t0):
                # xW GEMM for steps [t0, t0+ABLK) -> xw_d records
                x_tile = x_holder[t0 // XBLK]
                rel = t0 % XBLK
                x8 = [
                    bass.AP(
                        tensor=x_tile.tensor,
                        offset=x_tile.offset + (k * XBLK + rel) * nb,
                        ap=[x_tile.ap[0], [1, ABLK * nb]],
                    )
                    for k in range(KX)
                ]
                for q in range(4):
                    xws = gsb.tile([128, 2, NHALF], F16, tag="XWS", name="xws")
                    for n2 in range(2):
                        col = q * H + n2 * NHALF
                        xwp = pa.tile(
                            [128, 512], F32, tag="xwa", bufs=2, name="xwp"
                        )
                        for k in range(KX):
                            nc.tensor.matmul(
                                xwp[:, 0:NHALF],
                                x8[k],
                                w_sb[:, k, col : col + NHALF],
                                start=(k == 0),
                                stop=(k == KX - 1),
                            )
                        if has_b:
                            bq = b_sb[:, col : col + NHALF]
                            bq = bass.AP(
                                tensor=bq.tensor, offset=bq.offset,
                                ap=[[0, 128], [1, NHALF]],
                            )
                            nc.vector.tensor_add(
                                xws[:, n2, :], xwp[:, 0:NHALF], bq
                            )
                        elif cp_i[0] % 2 == 0:
                            nc.scalar.copy(
                                out=xws[:, n2, :], in_=xwp[:, 0:NHALF]
                            )
                        else:
                            nc.vector.tensor_copy(
                                xws[:, n2, :], xwp[:, 0:NHALF]
                            )
                        cp_i[0] += 1
                    nc.sync.dma_start(out=xw_d[t0 : t0 + ABLK, q], in_=xws)

            # prologue: stage 0 + blocks covering steps [0, LOOK)
            x_stage(0)
            for b0 in range(0, min(LOOK, t_steps), ABLK):
                xw_block(b0)

            # ---- initial state ----
            h0_sb = consts.tile([nb, H], F32)
            nc.sync.dma_start(out=h0_sb, in_=h0_d[:, :])
            c_prev = [None, None]
            for n2 in range(2):
                ct = state.tile([32 + nb, NHALF], F16, tag=f"C{n2}", name=f"C0_{n2}")
                nc.sync.dma_start(out=ct[32 : 32 + nb], in_=c0_d[:, n2, :])
                c_prev[n2] = ct

            h0_16 = consts.tile([nb, H], F16)
            nc.vector.tensor_copy(h0_16, h0_sb)
            ht_prev = state.tile([128, KH * nb], F16, tag="HT", name="ht_init")
            for n2 in range(2):
                ht0_ps = pt.tile([128, 3 * nb], F16, tag="tps", name="ht0_ps")
                for j in range(3):
                    ck = 3 * n2 + j
                    nc.tensor.transpose(
                        ht0_ps[:, j * nb : (j + 1) * nb],
                        h0_16[:, ck * 128 : (ck + 1) * 128],
                        ident16,
                    )
                nc.scalar.copy(
                    out=ht_prev[:, 3 * n2 * nb : (3 * n2 + 3) * nb], in_=ht0_ps
                )

            # ---- xw prefetch ring ----
            xw_tiles = {}

            def prefetch(t):
                xt = state.tile(
                    [4 * nb, 2, NHALF], F16, tag="XWT", bufs=PF + 1, name="xw_t"
                )
                nc.sync.dma_start(out=xt, in_=xw_d[t])
                xw_tiles[t] = xt

            for t in range(min(PF, t_steps)):
                prefetch(t)

            # ---- recurrence ----
            def u_wave(gt, n2, ks, ht_in):
                for k in ks:
                    for q in range(4):
                        col = q * H + n2 * NHALF
                        nc.tensor.matmul(
                            gt[32 * q : 32 * q + nb, 0:NHALF],
                            ht_in[:, k * nb : (k + 1) * nb],
                            u_sb[:, k, col : col + NHALF],
                            start=False,
                            stop=(k == KH - 1),
                            tile_position=(0, 32 * q),
                            skip_group_check=True,
                        )

            def inject(gt, n2, xw_t):
                nc.tensor.matmul(
                    gt[0:112, 0:NHALF],
                    sinj,
                    xw_t[:, n2, :],
                    start=True,
                    stop=False,
                    skip_group_check=True,
                )

            def tail(n2, oT, TC, ht_tile):
                # tanh(c)^T via PE, then h^T = o'^T * tc^T into stationary
                tcT = pt.tile([128, 3 * nb], F16, tag="tps", name=f"tcT_{n2}")
                for j in range(3):
                    nc.tensor.transpose(
                        tcT[:, j * nb : (j + 1) * nb],
                        TC[64 : 64 + nb, j * 128 : (j + 1) * 128],
                        idento[64 : 64 + nb],
                    )
                nc.vector.tensor_mul(
                    ht_tile[:, 3 * n2 * nb : (3 * n2 + 3) * nb], oT, tcT
                )

            half1_pend = None  # (oT1, TC1, ht_tile, t)

            for t in range(t_steps):
                xw_t = xw_tiles.pop(t)
                g0 = pg.tile([128, 512], F32, tag="gates0", name="g0")
                g1 = pg.tile([128, 512], F32, tag="gates1", name="g1")
                inject(g0, 0, xw_t)
                inject(g1, 1, xw_t)
                # k0-2 contract h[0:384] = half0's h of t-1: no tail1 dep
                u_wave(g0, 0, (0, 1, 2), ht_prev)
                u_wave(g1, 1, (0, 1, 2), ht_prev)
                # deferred half1 tail of step t-1 (ht[48:96] + hs DMA)
                if half1_pend is not None:
                    oT1p, TC1p, htp, tp = half1_pend
                    tail(1, oT1p, TC1p, htp)
                    nc.sync.dma_start(out=hs_d[tp, :, :], in_=htp)
                u_wave(g0, 0, (3, 4, 5), ht_prev)
                u_wave(g1, 1, (3, 4, 5), ht_prev)

                # h-independent PE filler: xW block + x staging lookahead
                if t % ABLK == 0 and t + LOOK < t_steps:
                    if (t + LOOK) % XBLK == 0:
                        x_stage(t + LOOK)
                    xw_block(t + LOOK)

                if t + PF < t_steps:
                    prefetch(t + PF)

                ht_new = state.tile([128, KH * nb], F16, tag="HT", name="ht_new")

                # ---- half 0 chain ----
                S20 = gsb.tile([112, NHALF], F16, tag="S20", name="S2_0")
                nc.scalar.activation(
                    out=S20, in_=g0[0:112, 0:NHALF], func=SIG, scale=sc
                )
                # o'^T early (off the c-chain)
                oT0ps = pt.tile([128, 3 * nb], F16, tag="tps", name="oT0_ps")
                for j in range(3):
                    nc.tensor.transpose(
                        oT0ps[:, j * nb : (j + 1) * nb],
                        S20[64 : 64 + nb, j * 128 : (j + 1) * 128],
                        idento[64 : 64 + nb],
                    )
                oT0 = ew.tile([128, 3 * nb], F16, tag="oT0", name="oT_0")
                nc.vector.tensor_copy(oT0, oT0ps)
                G0 = gsb.tile([nb, NHALF], F16, tag="G0", name="G_0")
                nc.vector.tensor_scalar(G0, S20[96:112], 2.0, -1.0, MULT, ADD)
                C0 = state.tile([32 + nb, NHALF], F16, tag="C0", name="C_0")
                nc.vector.tensor_mul(
                    C0[32 : 32 + nb], S20[32 : 32 + nb], c_prev[0][32 : 32 + nb]
                )
                T10 = ew.tile([32 + nb, NHALF], F16, tag="T10", name="T1_0")
                nc.vector.tensor_mul(T10[32 : 32 + nb], S20[0:nb], G0)
                nc.vector.tensor_add(
                    C0[32 : 32 + nb], C0[32 : 32 + nb], T10[32 : 32 + nb]
                )

                # half1 sigmoid + o'^T early (ACT queue: sig1 before tanh0)
                S21 = gsb.tile([112, NHALF], F16, tag="S21", name="S2_1")
                nc.scalar.activation(
                    out=S21, in_=g1[0:112, 0:NHALF], func=SIG, scale=sc
                )
                oT1ps = pt.tile([128, 3 * nb], F16, tag="tps", name="oT1_ps")
                for j in range(3):
                    nc.tensor.transpose(
                        oT1ps[:, j * nb : (j + 1) * nb],
                        S21[64 : 64 + nb, j * 128 : (j + 1) * 128],
                        idento[64 : 64 + nb],
                    )
                oT1 = ew.tile([128, 3 * nb], F16, tag="oT1", name="oT_1")
                nc.vector.tensor_copy(oT1, oT1ps)

                TC0 = ew.tile([64 + nb, NHALF], F16, tag="TC0", name="TC_0")
                nc.scalar.activation(
                    out=TC0[64 : 64 + nb], in_=C0[32 : 32 + nb], func=TANH
                )
                tail(0, oT0, TC0, ht_new)

                # ---- half 1 chain rest (tail deferred) ----
                G1 = gsb.tile([nb, NHALF], F16, tag="G1", name="G_1")
                nc.vector.tensor_scalar(G1, S21[96:112], 2.0, -1.0, MULT, ADD)
                C1 = state.tile([32 + nb, NHALF], F16, tag="C1", name="C_1")
                nc.vector.tensor_mul(
                    C1[32 : 32 + nb], S21[32 : 32 + nb], c_prev[1][32 : 32 + nb]
                )
                T11 = ew.tile([32 + nb, NHALF], F16, tag="T11", name="T1_1")
                nc.vector.tensor_mul(T11[32 : 32 + nb], S21[0:nb], G1)
                nc.vector.tensor_add(
                    C1[32 : 32 + nb], C1[32 : 32 + nb], T11[32 : 32 + nb]
                )
                TC1 = ew.tile([64 + nb, NHALF], F16, tag="TC1", name="TC_1")
                nc.scalar.activation(
                    out=TC1[64 : 64 + nb], in_=C1[32 : 32 + nb], func=TANH
                )

                nc.sync.dma_start(out=cs_d[:, t, 0], in_=C0[32 : 32 + nb])
                nc.sync.dma_start(out=cs_d[:, t, 1], in_=C1[32 : 32 + nb])
                c_prev = [C0, C1]
                half1_pend = (oT1, TC1, ht_new, t)
                ht_prev = ht_new

            oT1p, TC1p, htp, tp = half1_pend
            tail(1, oT1p, TC1p, htp)
            nc.sync.dma_start(out=hs_d[tp, :, :], in_=htp)

    nc.finalize()
    return nc


# Column permutation: reference gate order (i, f, g~, o) -> kernel (i, f, o, g~)
def _gate_perm():
    return np.concatenate(
        [np.arange(0, H), np.arange(H, 2 * H), np.arange(3 * H, 4 * H),
         np.arange(2 * H, 3 * H)]
    )


def _prep_core_inputs(input_, h0, c0, Wp, Up, bp, t_steps):
    nb = input_.shape[0]
    xT = np.ascontiguousarray(
        input_[:, :t_steps].transpose(1, 2, 0).reshape(t_steps, KX, 128, nb)
    )
    return {
        "xT": xT,
        "h0": np.ascontiguousarray(h0),
        "c0": np.ascontiguousarray(c0.reshape(nb, 2, NHALF).astype(np.float16)),
        "w": Wp,
        "u": Up,
        "b": bp,
    }


def run(input, hiddenState, cellState, W, U, b, t_steps=T, trace=False):
    input = np.asarray(input, np.float32)
    hiddenState = np.asarray(hiddenState, np.float32)
    cellState = np.asarray(cellState, np.float32)
    W = np.asarray(W, np.float32)
    U = np.asarray(U, np.float32)
    b = np.asarray(b, np.float32)

    perm = _gate_perm()
    Wp = np.ascontiguousarray(W[:, perm].reshape(KX, 128, 4 * H))
    Up = np.ascontiguousarray(U[:, perm].reshape(KH, 128, 4 * H))
    bp = np.ascontiguousarray(b[perm].reshape(1, 4 * H))
    has_b = bool(np.any(b))

    nc = build_lstm(NB, t_steps, has_b)
    in_maps = []
    for c in range(NCORES):
        bs = slice(c * NB, (c + 1) * NB)
        in_maps.append(
            _prep_core_inputs(
                input[bs], hiddenState[bs], cellState[bs], Wp, Up, bp, t_steps
            )
        )
    res = run_bass_kernel_spmd(
        nc, in_maps, core_ids=list(range(NCORES)), trace=trace
    )

    hs = np.empty((B, t_steps, H), np.float32)
    cs = np.empty((B, t_steps, H), np.float32)
    for c in range(NCORES):
        bs = slice(c * NB, (c + 1) * NB)
        ht = res.results[c]["hs"].astype(np.float32)  # [t, 128, 6*16]
        ht = ht.reshape(t_steps, 128, KH, NB)
        hs[bs] = ht.transpose(3, 0, 2, 1).reshape(NB, t_steps, H)
        cs[bs] = res.results[c]["cs"].astype(np.float32).reshape(NB, t_steps, H)
    return (hs, cs), res


def kernel(input, hiddenState, cellState, W, U, b):
    (hs, cs), _ = run(input, hiddenState, cellState, W, U, b)
    return hs, cs
